# revision 60
# baseline (speedup 1.0000x reference)
"""Bass/Trainium2 kernel for nn_BipartiteSoftMatching (8 cores, batch-parallel).

Since r = t//2 the argsort in the reference is irrelevant: src_idx is a full
permutation and unm_idx is empty.  Per batch element the computation reduces to
  m = metric / ||metric||;  scores = m_even @ m_odd^T
  node_idx[i] = argmax_j scores[i, j]
  dst_out[j]  = (x_odd[j] + sum_{i: node_idx[i]=j} x_even[i]) / (1 + count[j])
  out[2j+1]   = dst_out[j];   out[2i] = dst_out[node_idx[i]]

Only argmax_j matters, so the even side of metric is left unnormalized (a
positive per-row scale cannot change the argmax); only the odd side is
normalized.

Scheduling notes (engine queues are in-order; indirect DMA writes to the
same DRAM tensor serialize on ~3us completion round-trips):
  - the 16 permutation-table scatters are interleaved into phase B with a
    one-pair lag, hiding each scatter's round-trip behind the ~10us/pair
    argmax cadence;
  - vector-queue ops never wait on a result another engine produced less
    than a pair-period ago (no head-of-line stalls ahead of max8/find);
  - even output rows go through a DRAM staging buffer with read-only
    indirect gathers (reads pipeline; scatters would serialize).

Hardware mapping:
  - scores: fp32 matmuls (exact LOW_HIGH mode; f32r would flip argmaxes),
    K=64 pairs row-packed via tile_position. PSUM drained by scalar,
    argmax = DVE max8 + find_index8.
  - bucket permutation: slot = 256*bucket + cross-tile-count + within-tile
    rank. Rank via a strict-upper-triangular matmul prefix of the 16-bucket
    one-hot; cross-tile counts via an incrementally updated [1,16]
    histogram.
  - scatter-add: per destination j-tile, one dual-row bounds-checked
    indirect gather (pad slots skipped) + 2 one-hot f32r matmuls (single
    pass; ~1e-4 rel err, well within tolerance). A ones-column matmul in a
    separate PSUM bank yields the counts.
"""

import numpy as np

import concourse.bacc as bacc
import concourse.bass as bass
import concourse.mybir as mybir
import concourse.tile as tile
from concourse.bass import IndirectOffsetOnAxis
from concourse.bass_utils import run_bass_kernel_spmd
from concourse.masks import make_identity

F32 = mybir.dt.float32
F32R = mybir.dt.float32r
U32 = mybir.dt.uint32
I32 = mybir.dt.int32
OP = mybir.AluOpType
AF = mybir.ActivationFunctionType

N, T, CM, CX = 8, 4096, 64, 768
P = 128
T1 = T // 2          # 2048 tokens per side
TI = T1 // P         # 16 i-tiles (even side)
TJ = T1 // P         # 16 j-tiles (odd side)
NSLOT = 2 * T1       # 16 buckets x 256 slots
NU = NSLOT // P      # 32 slot-tiles
BIG = 1 << 20        # pad marker in the permutation table

_CACHE = {}


def _build(debug=False):
    nc = bacc.Bacc("TRN2", target_bir_lowering=False, num_devices=N)
    metric_in = nc.declare_dram_parameter("metric", [T, CM], F32, isOutput=False)
    x_in = nc.declare_dram_parameter("x", [T, CX], F32, isOutput=False)
    out = nc.declare_dram_parameter("out", [T, CX], F32, isOutput=True)
    if debug:
        perm8 = nc.declare_dram_parameter("perm8", [NSLOT, 2], I32,
                                          isOutput=True)
        prdbg = nc.declare_dram_parameter("prdbg", [P, TI * 2], I32,
                                          isOutput=True)
        sldbg = nc.declare_dram_parameter("sldbg", [P, TI], I32,
                                          isOutput=True)
        cndbg = nc.declare_dram_parameter("cndbg", [P, TJ], F32,
                                          isOutput=True)
    else:
        perm8 = nc.dram_tensor("perm8", [NSLOT, 2], I32)
    dstbuf = nc.dram_tensor("dstbuf", [T1, CX], F32)

    # token = (t*128 + p)*2 + e
    m_pv = metric_in[:].rearrange("(t p e) c -> e p t c", p=P, e=2)
    x_pv_odd = x_in[:].rearrange("(t p e) c -> e p t c", p=P, e=2)[1]
    out_r = out[:].rearrange("(t p e) c -> e t p c", p=P, e=2)
    out_pv = out[:].rearrange("(t p e) c -> e p t c", p=P, e=2)
    perm_pv = perm8[:].rearrange("(u p) w -> p u w", p=P)

    with tile.TileContext(nc, num_cores=N) as tc:
        with tc.tile_pool(name="const", bufs=1) as cp:
            ident = cp.tile([P, P], F32)
            make_identity(nc, ident[:])
            iota16 = cp.tile([P, 16], F32)
            nc.gpsimd.iota(iota16[:], pattern=[[1, 16]], base=0,
                           channel_multiplier=0,
                           allow_small_or_imprecise_dtypes=True)
            ones128 = cp.tile([P, P], F32)
            nc.vector.memset(ones128[:], 1.0)
            # UT[p, f] = 1.0 if f > p else 0.0 (strict upper triangle):
            # within-tile rank prefix matmul lhsT
            ut_i = cp.tile([P, P], I32)
            nc.gpsimd.iota(ut_i[:], pattern=[[1, P]], base=0,
                           channel_multiplier=-1)
            utm = cp.tile([P, P], F32)
            nc.vector.tensor_scalar(utm[:], ut_i[:], 0, None, op0=OP.is_gt)
            # jrowP[p, f] = f for the one-hot builds
            jrowP = cp.tile([P, P], F32)
            nc.gpsimd.iota(jrowP[:], pattern=[[1, P]], base=0,
                           channel_multiplier=0,
                           allow_small_or_imprecise_dtypes=True)
            # xrow[p, t] = 2p + 256t = DRAM row of even token (t*128+p)
            xrow_i32 = cp.tile([P, TI], I32)
            nc.gpsimd.iota(xrow_i32[:], pattern=[[256, TI]], base=0,
                           channel_multiplier=2)
            bigpat = cp.tile([P, 2 * NSLOT // P], I32)
            nc.vector.memset(bigpat[:], BIG)
            ones2r = cp.tile([P, 2], F32R)
            nc.vector.tensor_copy(ones2r[:], ones128[:, 0:2])
            zcol = cp.tile([P, 1], F32)
            nc.vector.memset(zcol[:], 0.0)

            xodd_all = cp.tile([P, TJ * CX], F32)
            dst_all = cp.tile([P, TJ * CX], F32R)
            pr_all = cp.tile([P, TI * 2], I32)
            # even columns = xrow (constant), set once
            nc.vector.tensor_copy(
                pr_all[:].rearrange("p (t w) -> p t w", w=2)[:, :, 0],
                xrow_i32[:])
            slot_i32 = cp.tile([P, TI], I32)
            mi_all = cp.tile([P, TI * 8], U32)
            bf_all = cp.tile([P, TI], F32)
            wc_all = cp.tile([P, TI], F32)
            oh_all = cp.tile([P, TI * 16], F32)
            offs = cp.tile([P, TI], I32)
            crun = cp.tile([1, 16], F32)       # running bucket counts
            nc.vector.memset(crun[:], 0.0)

            # pre-fill the permutation table with the OOB marker
            nc.sync.dma_start(out=perm8[:].rearrange("(p u) w -> p (u w)", p=P),
                              in_=bigpat[:])

            with tc.tile_pool(name="work", bufs=1) as wp:
                aTpk = wp.tile([P, T1 // 2], F32)
                bTpk4 = [wp.tile([P, 512], F32, name=f"bTpk{q}") for q in range(4)]
                me = wp.tile([P, TI * CM], F32)
                mo = wp.tile([P, TI * CM], F32)

                # mo first, chunked: the normalize stream (B's critical
                # path) starts after the first 4 tiles land
                for ch in range(4):
                    nc.sync.dma_start(
                        out=mo[:, ch * 4 * CM:(ch + 1) * 4 * CM].rearrange(
                            "p (t c) -> p t c", c=CM),
                        in_=m_pv[1][:, ch * 4:(ch + 1) * 4, :])
                nc.sync.dma_start(out=me[:].rearrange("p (t c) -> p t c", c=CM),
                                  in_=m_pv[0])
                # prefetch x_odd early; consumed in phase C
                nc.sync.dma_start(
                    out=xodd_all[:].rearrange("p (t c) -> p t c", c=CX),
                    in_=x_pv_odd)

                # ---- Phase A: pack transposed operands ----
                with tc.tile_pool(name="pA", bufs=3) as pa, \
                     tc.tile_pool(name="psA", bufs=3, space="PSUM") as psa:
                    # even side: raw pair transpose (no normalization --
                    # row scale does not affect argmax). Pair 0 first (it
                    # gates B's first matmul), the rest after the odd side
                    # so they overlap phase B.
                    def even_pair(q):
                        pst = psa.tile([P, P], F32, tag="tp", space="PSUM")
                        nc.tensor.transpose(pst[:], me[:, 2 * q * CM:(2 * q + 2) * CM],
                                            ident[:])
                        nc.scalar.copy(aTpk[:, q * P:(q + 1) * P], pst[:])

                    even_pair(0)
                    # odd side: normalize, duplicate, transpose (per-tile
                    # chains pipeline; no batch barrier)
                    for t in range(TI):
                        mt = mo[:, t * CM:(t + 1) * CM]
                        sq = pa.tile([P, CM], F32, tag="sq")
                        ssum = pa.tile([P, 1], F32, tag="ss")
                        nc.scalar.activation(sq[:], mt, AF.Square,
                                             accum_out=ssum[:])
                        nrm = pa.tile([P, 1], F32, tag="nr")
                        nc.scalar.sqrt(nrm[:], ssum[:])
                        rnm = pa.tile([P, 1], F32, tag="rn")
                        nc.vector.reciprocal(rnm[:], nrm[:])
                        nm2 = pa.tile([P, 2 * CM], F32, tag="nm")
                        nc.vector.tensor_scalar_mul(nm2[:, 0:CM], mt, rnm[:, 0:1])
                        nc.vector.tensor_copy(nm2[:, CM:2 * CM], nm2[:, 0:CM])
                        pst = psa.tile([P, P], F32, tag="tp", space="PSUM")
                        nc.tensor.transpose(pst[:], nm2[:], ident[:])
                        blk = bTpk4[t // 4][:, (t % 4) * P:(t % 4 + 1) * P]
                        nc.scalar.copy(blk, pst[:])
                    for q in range(1, TI // 2):
                        even_pair(q)

                # ---- Phase B: scores + argmax + slots + perm scatters ----
                with tc.tile_pool(name="pB", bufs=3) as pb, \
                     tc.tile_pool(name="pS", bufs=2) as psb_s, \
                     tc.tile_pool(name="psB", bufs=2, space="PSUM") as psb, \
                     tc.tile_pool(name="psR", bufs=3, space="PSUM") as psr:

                    pend = []          # (i, oh, pfx) awaiting slot work

                    def slot_work(i, oh, pfx):
                        """within-tile rank, cross-tile count, slot, scatter
                        for a tile whose argmax finished a pair-period ago"""
                        # W[p] = #{p' < p with same bucket}
                        s1 = pb.tile([P, 16], F32, tag="s1")
                        nc.vector.scalar_tensor_tensor(
                            out=s1[:], in0=oh, scalar=1.0, in1=pfx[:],
                            op0=OP.mult, op1=OP.mult,
                            accum_out=wc_all[:, i:i + 1])
                        # cross-tile count so far (crun before this tile)
                        cb = pb.tile([P, 16], F32, tag="cb")
                        nc.gpsimd.partition_broadcast(cb[:], crun[:])
                        ctv = pb.tile([P, 1], F32, tag="ctv")
                        s2 = pb.tile([P, 16], F32, tag="s2")
                        nc.vector.scalar_tensor_tensor(
                            out=s2[:], in0=oh, scalar=1.0, in1=cb[:],
                            op0=OP.mult, op1=OP.mult, accum_out=ctv[:])
                        # crun += hist(this tile)
                        nc.vector.tensor_add(crun[:], crun[:],
                                             oh_hist[i][:])
                        # slot = 256*b + ctv + W
                        sf = pb.tile([P, 1], F32, tag="sf")
                        nc.vector.scalar_tensor_tensor(
                            out=sf[:], in0=bf_all[:, i:i + 1], scalar=256.0,
                            in1=ctv[:], op0=OP.mult, op1=OP.add)
                        nc.vector.tensor_tensor(out=sf[:], in0=sf[:],
                                                in1=wc_all[:, i:i + 1],
                                                op=OP.add)
                        nc.gpsimd.tensor_copy(slot_i32[:, i:i + 1], sf[:])
                        nc.gpsimd.tensor_copy(pr_all[:, 2 * i + 1:2 * i + 2],
                                              mi_all[:, 8 * i:8 * i + 1])
                        nc.gpsimd.indirect_dma_start(
                            out=perm8[:], in_=pr_all[:, 2 * i:2 * i + 2],
                            in_offset=None,
                            out_offset=IndirectOffsetOnAxis(
                                ap=slot_i32[:, i:i + 1], axis=0))

                    oh_hist = {}
                    for ii in range(TI // 2):
                        i0, i1 = 2 * ii, 2 * ii + 1
                        ssb0 = psb_s.tile([P, T1], F32, tag="scores0")
                        ssb1 = psb_s.tile([P, T1], F32, tag="scores1")
                        for c in range(4):
                            nj = c * 512
                            ps0 = psb.tile([P, 512], F32, tag="ps0",
                                           space="PSUM")
                            ps1 = psb.tile([P, 512], F32, tag="ps1",
                                           space="PSUM")
                            nc.tensor.matmul(ps0[:],
                                             aTpk[0:CM, ii * P:(ii + 1) * P],
                                             bTpk4[c][0:CM, :],
                                             start=True, stop=True,
                                             tile_position=(0, 0))
                            nc.tensor.matmul(ps1[:],
                                             aTpk[CM:P, ii * P:(ii + 1) * P],
                                             bTpk4[c][CM:P, :],
                                             start=True, stop=True,
                                             tile_position=(64, 0))
                            nc.scalar.copy(ssb0[:, nj:nj + 512], ps0[:])
                            nc.scalar.copy(ssb1[:, nj:nj + 512], ps1[:])
                        for i, ssb in ((i0, ssb0), (i1, ssb1)):
                            mi8 = mi_all[:, 8 * i:8 * i + 8]
                            mx8 = pb.tile([P, 8], F32, tag="mx8")
                            nc.vector.max(out=mx8[:], in_=ssb[:])
                            nc.vector.max_index(out=mi8, in_max=mx8[:],
                                                in_values=ssb[:])
                            bu = pb.tile([P, 1], U32, tag="bu")
                            nc.vector.tensor_scalar(
                                bu[:], mi8[:, 0:1], 7, None,
                                op0=OP.logical_shift_right)
                            # cast off the vector queue (gpsimd is idle)
                            nc.gpsimd.tensor_copy(bf_all[:, i:i + 1], bu[:])
                        for i in (i0, i1):
                            oh = oh_all[:, 16 * i:16 * i + 16]
                            nc.vector.tensor_scalar(oh, iota16[:],
                                                    bf_all[:, i:i + 1],
                                                    None, op0=OP.is_equal)
                            pfx = psr.tile([P, 16], F32, tag="pfx",
                                           space="PSUM")
                            nc.tensor.matmul(pfx[:], utm[:], oh,
                                             start=True, stop=True)
                            hp = psr.tile([1, 16], F32, tag="hp",
                                          space="PSUM", bufs=1)
                            nc.tensor.matmul(hp[:], ones128[:, 0:1], oh,
                                             start=True, stop=True)
                            hrow = pb.tile([1, 16], F32, tag="hr")
                            nc.scalar.copy(hrow[:], hp[:])
                            oh_hist[i] = hrow
                            pend.append((i, oh, pfx))
                        # slot work for the PREVIOUS pair: its pfx matmuls
                        # and casts are long done -> no vector stalls
                        if ii > 0:
                            slot_work(*pend.pop(0))
                            slot_work(*pend.pop(0))
                    slot_work(*pend.pop(0))
                    slot_work(*pend.pop(0))
                    # gather offsets for the final even-row pass
                    nc.vector.tensor_copy(
                        offs[:],
                        mi_all[:].rearrange("p (t e) -> p t e", e=8)[:, :, 0])

            if debug:
                nc.sync.dma_start(out=prdbg[:], in_=pr_all[:])
                nc.sync.dma_start(out=sldbg[:], in_=slot_i32[:])

            # ---- Phase C: bucketed one-hot scatter matmul ----
            with tc.tile_pool(name="pq", bufs=1) as pqp:
                qrow = pqp.tile([P, NU], I32)
                nc.sync.dma_start(out=qrow[:], in_=perm_pv[:, :, 0])
                idxg_i = pqp.tile([P, NU], I32)
                nc.sync.dma_start(out=idxg_i[:], in_=perm_pv[:, :, 1])
                idxg_f = pqp.tile([P, NU], F32)
                nc.vector.tensor_copy(idxg_f[:], idxg_i[:])
                # clamped offsets for the first ring generation of gather
                # buffers (fills every row so later bounds-checked gathers
                # can safely skip pad rows, leaving stale finite values)
                qclmp = pqp.tile([P, 12], I32)
                nc.vector.tensor_scalar(qclmp[:], qrow[:, 0:12], T - 2, None,
                                        op0=OP.min)

                with tc.tile_pool(name="pC", bufs=3) as pcs, \
                     tc.tile_pool(name="pD", bufs=3) as pd, \
                     tc.tile_pool(name="psC", bufs=3, space="PSUM") as psc, \
                     tc.tile_pool(name="psN", bufs=2, space="PSUM") as psn_p:
                    # deep gather ring: the 32 row-gathers stream on gpsimd
                    # well ahead of the consuming matmuls
                    for jt in range(TJ):
                        psj = psc.tile([P, CX], F32, tag="sp", space="PSUM")
                        psn = psn_p.tile([P, 2], F32, tag="sn", space="PSUM")
                        xg2 = pcs.tile([P, 2 * CX], F32R, tag="xg", bufs=6)
                        for k in range(2):
                            u = 2 * jt + k
                            if jt < 6:
                                # clamped, unchecked: fills all 128 rows
                                nc.gpsimd.indirect_dma_start(
                                    out=xg2[:, k * CX:(k + 1) * CX],
                                    out_offset=None,
                                    in_=x_in[:].bitcast(F32R),
                                    in_offset=IndirectOffsetOnAxis(
                                        ap=qclmp[:, u:u + 1], axis=0))
                            else:
                                nc.gpsimd.indirect_dma_start(
                                    out=xg2[:, k * CX:(k + 1) * CX],
                                    out_offset=None,
                                    in_=x_in[:].bitcast(F32R),
                                    in_offset=IndirectOffsetOnAxis(
                                        ap=qrow[:, u:u + 1], axis=0),
                                    bounds_check=T - 1, oob_is_err=False)
                        for k in range(2):
                            u = 2 * jt + k
                            eqr = pcs.tile([P, P], F32R, tag="eq")
                            nc.vector.scalar_tensor_tensor(
                                out=eqr[:],
                                in0=idxg_f[:, u:u + 1].to_broadcast([P, P]),
                                scalar=float(-128 * jt),
                                in1=jrowP[:],
                                op0=OP.add, op1=OP.is_equal)
                            first, last = (k == 0), (k == 1)
                            nc.tensor.matmul(psn[:], eqr[:], ones2r[:],
                                             start=first, stop=last)
                            for lo_, hi_ in ((0, 512), (512, CX)):
                                nc.tensor.matmul(
                                    psj[:, lo_:hi_], eqr[:],
                                    xg2[:, k * CX + lo_:k * CX + hi_],
                                    start=first, stop=last)
                        xo = xodd_all[:, jt * CX:(jt + 1) * CX]
                        cnt1 = pd.tile([P, 1], F32, tag="c1")
                        nc.vector.tensor_scalar_add(cnt1[:], psn[:, 0:1], 1.0)
                        if debug:
                            nc.scalar.dma_start(out=cndbg[:, jt:jt + 1],
                                                in_=cnt1[:])
                        inv = pd.tile([P, 1], F32, tag="iv")
                        nc.vector.reciprocal(inv[:], cnt1[:])
                        dsum = pd.tile([P, CX], F32, tag="dsum")
                        nc.vector.tensor_add(dsum[:], xo, psj[:, 0:CX])
                        dst = dst_all[:, jt * CX:(jt + 1) * CX]
                        nc.scalar.mul(dst, dsum[:], inv[:, 0:1])
                        nc.sync.dma_start(out=out_r[1, jt].bitcast(F32R),
                                          in_=dst)
                        nc.scalar.dma_start(
                            out=dstbuf[jt * P:(jt + 1) * P, :],
                            in_=dst.bitcast(F32))

                # ---- Phase D: even rows via read-only gathers ----
                with tc.tile_pool(name="pG", bufs=4) as pg:
                    for q in range(TI // 4):
                        gb = pg.tile([P, 4 * CX], F32, tag="gb")
                        for k in range(4):
                            i = 4 * q + k
                            nc.gpsimd.indirect_dma_start(
                                out=gb[:, k * CX:(k + 1) * CX],
                                out_offset=None,
                                in_=dstbuf[:],
                                in_offset=IndirectOffsetOnAxis(
                                    ap=offs[:, i:i + 1], axis=0))
                        nc.sync.dma_start(
                            out=out_pv[0][:, 4 * q:4 * q + 4, :],
                            in_=gb[:].rearrange("p (t c) -> p t c", c=CX))

    nc.compile()
    return nc


def kernel(metric: np.ndarray, x: np.ndarray) -> np.ndarray:
    if "nc" not in _CACHE:
        _CACHE["nc"] = _build()
    nc = _CACHE["nc"]
    metric = np.ascontiguousarray(np.asarray(metric, dtype=np.float32))
    x = np.ascontiguousarray(np.asarray(x, dtype=np.float32))
    in_maps = [{"metric": metric[c], "x": x[c]} for c in range(N)]
    res = run_bass_kernel_spmd(nc, in_maps, list(range(N)))
    return np.stack([res.results[c]["out"] for c in range(N)], axis=0)
